# revision 28
# baseline (speedup 1.0000x reference)
"""Multi-head attention (B=4, S=2048, D=1024, H=16, causal) on 8 TRN2 NeuronCores.

Sharding: batch x head-group (Megatron).  Core c handles batch c//2 and head
group c%2 (8 heads = 512 of the 1024 hidden dims).  w_q/w_k/w_v are
column-parallel, w_o row-parallel; the two partial outputs per batch are summed
on the host during unsharding.

Device kernel (per core, all matmuls bf16, fp32 accumulation):
  - inputs stream over 4 HWDGE rings (sync/gpsimd/vector/scalar) in
    consumption order: (wq, xq, wk, xk-tier0) interleaved k-major, then
    xv-tier0 + wv + wo, then the remaining k/v column tiers.  All x buffers
    are independent (no SBUF slot reuse), so no DMA ever WAR-blocks behind
    compute.  This pulls the first softmax exp from ~61us (v1) to ~30us --
    the exp stream on ScalarE (~165us of work, exp is ScalarE-only at
    1 elem/cycle/lane) is the second critical resource after the PE, so its
    span must start as early as possible and never starve late.
  - ~10 warm-up matmuls on a constant tile run while the first DMAs land so
    the PE HAM clock-gate reaches 8/8 before the projection wavefront
  - qproj ot0/ot1 as a k-outer wavefront over 8 open PSUM chains, consuming
    each xq k-tile as it lands
  - attention units (qc, hp) run exp-heavy-first after a minimal warmup:
    (0,0) and (0,1) with batched scores while DMA lands, then qc3/qc2 units
    mid-kernel while fill inventory (remaining q/k/v projections, o-proj of
    completed chunks) keeps the PE busy, ending with exp-light qc0 units so
    the tail is PE-dense
  - scoresT[k,q] = kT.T @ qT per head, two heads row-packed on the PE array
    (64-contraction matmuls at base partitions 0/64 run concurrently)
  - exp on ScalarE (scores are O(1): no max subtraction needed); causal
    masking via trimmed k-tiles + lower-tri mask multiply on the diagonal
  - attn@V stationary = [ones | V] per (tile, head) stitched via a strided
    AP over a deduplicated V buffer (ones stored once, 16KB SBUF saved), so
    the softmax denominator accumulates in PSUM partitions 0:64 for free
  - softmax denominator reciprocal on DVE (reciprocal_approx_fast)
  - k/v/q projection chains emit lazily (first unit that needs them,
    prefetched ~2 j-steps early) so no engine FIFO blocks on work behind it
  - b_q added on qT evacuation, b_k dropped (cancels in softmax), b_v folded
    into b_o on host
"""

import os
import sys

for _p in ("/opt/trn_rl_repo",):
    if _p not in sys.path and os.path.isdir(_p):
        sys.path.insert(0, _p)

from contextlib import ExitStack

import ml_dtypes
import numpy as np

import concourse.bass as bass
import concourse.tile as tile
from concourse import bacc, mybir
from concourse import bass_utils

BF16 = ml_dtypes.bfloat16

B = 4
S = 2048
D = 1024
H = 16
DK = 64
NCORES = 8
DL = D // 2  # local (per head-group) hidden dims = 512
NHP = 4  # head pairs per core
KT = D // 128  # contraction tiles over model dim = 8
TT = S // 128  # token tiles = 16
QC = S // 512  # query chunks of 512 = 4

FP32 = mybir.dt.float32
DTBF = mybir.dt.bfloat16


def _emit(nc, causal: bool):
    xq = nc.dram_tensor("xq_t", [D, S], DTBF, kind="ExternalInput").ap()
    xk = nc.dram_tensor("xk_t", [D, S], DTBF, kind="ExternalInput").ap()
    xv = nc.dram_tensor("xv_t", [D, S], DTBF, kind="ExternalInput").ap()
    wq_t = nc.dram_tensor("wq_p", [128, KT, DL], DTBF, kind="ExternalInput").ap()
    wk_t = nc.dram_tensor("wk_p", [128, KT, DL], DTBF, kind="ExternalInput").ap()
    wv_t = nc.dram_tensor("wv_p", [128, KT, DL], DTBF, kind="ExternalInput").ap()
    wo_t = nc.dram_tensor("wo_p", [128, NHP, D], DTBF, kind="ExternalInput").ap()
    bq_t = nc.dram_tensor("bq_t", [128, 4], FP32, kind="ExternalInput").ap()
    bo_t = nc.dram_tensor("bo_t", [128, 8], FP32, kind="ExternalInput").ap()
    # bf16 output partials: the two per-batch partials are upcast and summed
    # on the host; the added quantization (~3e-3 rel) is well inside budget
    out_pt = nc.dram_tensor("out_pt", [D, S], DTBF, kind="ExternalOutput").ap()

    EXP = mybir.ActivationFunctionType.Exp

    with tile.TileContext(nc) as tc, ExitStack() as ctx:
        consts = ctx.enter_context(tc.tile_pool(name="consts", bufs=1))
        xtk_pool = ctx.enter_context(tc.tile_pool(name="xtk", bufs=2))
        et_pool = ctx.enter_context(tc.tile_pool(name="et", bufs=4))
        rc_pool = ctx.enter_context(tc.tile_pool(name="rc", bufs=1))
        out_pool = ctx.enter_context(tc.tile_pool(name="osb", bufs=2))
        ps_s = ctx.enter_context(tc.tile_pool(name="ps_s", bufs=2, space="PSUM"))
        ps_acc = ctx.enter_context(tc.tile_pool(name="ps_acc", bufs=2, space="PSUM"))
        ps_op = ctx.enter_context(tc.tile_pool(name="ps_op", bufs=2, space="PSUM"))

        # ---- constant tiles -------------------------------------------------
        # lower-triangle-inclusive (k <= q) binary mask for diagonal tiles,
        # replicated for both heads of a pair.  Emitted FIRST on gpsimd (its
        # ring DMAs follow) -- it also feeds the HAM warm-up matmuls.
        tri_sb = consts.tile([128, 2, 128], DTBF)
        nc.gpsimd.memset(tri_sb[:], 1.0)
        for h2 in range(2):
            nc.gpsimd.affine_select(
                out=tri_sb[:, h2, :],
                in_=tri_sb[:, h2, :],
                compare_op=mybir.AluOpType.is_ge,
                fill=0.0,
                base=0,
                pattern=[[1, 128]],
                channel_multiplier=-1,
            )

        # ---- HAM warm-up: keep the PE busy from ~7us (preamble end) so the
        # clock gate hits 8/8 before the first real matmul instead of ~25us in
        dummy_ps = ps_op.tile([128, 512], FP32, tag="op", name="dummy")
        for _ in range(10):
            nc.tensor.matmul(
                dummy_ps[:, 0:128], tri_sb[:, 0, :], tri_sb[:, 1, :],
                start=True, stop=True,
            )

        # ---- SBUF buffers (all independent, no slot reuse) ------------------
        bq_sb = consts.tile([128, 4], FP32)
        wq_sb = consts.tile([128, KT, DL], DTBF)
        wk_sb = consts.tile([128, KT, DL], DTBF)
        wv_sb = consts.tile([128, KT, DL], DTBF)
        wo_sb = consts.tile([128, NHP, D], DTBF)
        bo_sb = consts.tile([128, 8], FP32)
        xt = consts.tile([128, KT, S], DTBF)          # q input, transposed
        # k input: 2-slot rotating pool (tier t -> slot t%2).  Pool rotation
        # gives the tier-2/3 DMAs a tracked WAR on the tier-0/1 kproj reads;
        # kproj of tiers 0/1 is forced into the warmup phase so those DMAs
        # flow mid-kernel.
        xtk_t = [
            xtk_pool.tile([128, KT, 512], DTBF, tag="xtk", name=f"xtk{t}")
            for t in range(4)
        ]
        xtv0 = consts.tile([128, KT, 512], DTBF)      # v input, column tier 0
        xtvr = consts.tile([128, KT, 3, 512], DTBF)   # v input, tiers 1..3

        qT_sb = consts.tile([128, NHP, S], DTBF)
        kT_sb = consts.tile([128, NHP, S], DTBF)
        # [ones(32) | V(64)] per (token-tile, head): the attn@V stationary is
        # 96 columns; the softmax denominator accumulates in PSUM partitions
        # 0:32 (32 identical rows -- enough to broadcast-normalize the two
        # 32-row halves of V), V rows land in partitions 32:96.
        vp_sb = consts.tile([128, TT, 8, 96], DTBF)
        a_sbs = [consts.tile([128, S], DTBF, name=f"a_sb{hp}") for hp in range(NHP)]

        # ---- input DMAs: consumption order over 4 rings ---------------------
        # DMA-capable queues: sync (SP), scalar (Activation), gpsimd
        R4 = [nc.sync, nc.gpsimd, nc.scalar]
        R3 = [nc.sync, nc.gpsimd, nc.scalar]
        R2 = [nc.sync, nc.gpsimd]
        _ri = [0]

        def ring(rl):
            r = rl[_ri[0] % len(rl)]
            _ri[0] += 1
            return r

        nc.scalar.dma_start(bq_sb[:], bq_t[:])
        # phase A+B: q-projection stream + first k tier, k-major interleave
        for k in range(KT):
            ring(R4).dma_start(wq_sb[:, k, :], wq_t[:, k, :])
            ring(R4).dma_start(xt[:, k, 0:1024], xq[k * 128:(k + 1) * 128, 0:1024])
            ring(R4).dma_start(xt[:, k, 1024:2048], xq[k * 128:(k + 1) * 128, 1024:2048])
            ring(R4).dma_start(wk_sb[:, k, :], wk_t[:, k, :])
            ring(R4).dma_start(xtk_t[0][:, k, :], xk[k * 128:(k + 1) * 128, 0:512])
        # phase C: v tier 0 + wv + wo + biases on sync+gpsimd only -- the
        # scalar queue must be free for exps from ~28us on.
        for k in range(KT):
            ring(R2).dma_start(wv_sb[:, k, :], wv_t[:, k, :])
            ring(R2).dma_start(xtv0[:, k, :], xv[k * 128:(k + 1) * 128, 0:512])
        for hp in range(NHP):
            ring(R2).dma_start(wo_sb[:, hp, :], wo_t[:, hp, :])
        nc.gpsimd.dma_start(bo_sb[:], bo_t[:])
        # phase D/E: remaining k/v column tiers interleaved on sync+gpsimd
        # only -- the scalar queue must drain its DMAs before the first exp,
        # so it carries nothing past phase A+B.
        for t in range(3):
            for k in range(KT):
                ring(R2).dma_start(
                    xtk_t[t + 1][:, k, :],
                    xk[k * 128:(k + 1) * 128, (t + 1) * 512:(t + 2) * 512],
                )
            for k in range(KT):
                ring(R2).dma_start(
                    xtvr[:, k, t, :],
                    xv[k * 128:(k + 1) * 128, (t + 1) * 512:(t + 2) * 512],
                )

        for h in range(8):
            nc.vector.memset(vp_sb[:, :, h, 0:32], 1.0)

        # ---- q-projection ---------------------------------------------------
        # ot0+ot1 as a k-outer wavefront over 8 open PSUM chains: the PE
        # consumes each xq k-tile as it lands instead of waiting for the
        # full tensor
        wf_s = [ps_s.tile([128, 2, 512], FP32, tag="ps_s", name=f"wfs{i}") for i in range(2)]
        wf_o = [ps_op.tile([128, 512], FP32, tag="op", name=f"wfo{i}") for i in range(2)]
        wf_a = ps_acc.tile([128, 2, 512], FP32, tag="acc", name="wfa", bufs=1)
        for k in range(KT):
            for c in range(4):
                nc.tensor.matmul(
                    wf_s[c // 2][:, c % 2, :],
                    wq_sb[:, k, 0:128],
                    xt[:, k, c * 512:(c + 1) * 512],
                    start=(k == 0),
                    stop=(k == KT - 1),
                )
            for c in range(4):
                dst = wf_o[c][:] if c < 2 else wf_a[:, c - 2, :]
                nc.tensor.matmul(
                    dst,
                    wq_sb[:, k, 128:256],
                    xt[:, k, c * 512:(c + 1) * 512],
                    start=(k == 0),
                    stop=(k == KT - 1),
                )
        for c in range(4):
            nc.vector.tensor_scalar_add(
                qT_sb[:, 0, c * 512:(c + 1) * 512], wf_s[c // 2][:, c % 2, :], bq_sb[:, 0:1]
            )
        for c in range(4):
            src = wf_o[c][:] if c < 2 else wf_a[:, c - 2, :]
            nc.vector.tensor_scalar_add(
                qT_sb[:, 1, c * 512:(c + 1) * 512], src, bq_sb[:, 1:2]
            )

        _qdone = {(0, t) for t in range(4)} | {(1, t) for t in range(4)}

        def qproj_chain(ot, tc4):
            if (ot, tc4) in _qdone:
                return
            _qdone.add((ot, tc4))
            ps = ps_op.tile([128, 512], FP32, tag="op", name="ps")
            for k in range(KT):
                nc.tensor.matmul(
                    ps[:],
                    wq_sb[:, k, ot * 128:(ot + 1) * 128],
                    xt[:, k, tc4 * 512:(tc4 + 1) * 512],
                    start=(k == 0),
                    stop=(k == KT - 1),
                )
            nc.vector.tensor_scalar_add(
                qT_sb[:, ot, tc4 * 512:(tc4 + 1) * 512], ps[:], bq_sb[:, ot:ot + 1]
            )

        _kdone = set()

        def kproj_chain(ot, tc4):
            if (ot, tc4) in _kdone:
                return
            _kdone.add((ot, tc4))
            ps = ps_op.tile([128, 512], FP32, tag="op", name="ps")
            for k in range(KT):
                src = xtk_t[tc4][:, k, :]
                nc.tensor.matmul(
                    ps[:],
                    wk_sb[:, k, ot * 128:(ot + 1) * 128],
                    src,
                    start=(k == 0),
                    stop=(k == KT - 1),
                )
            nc.vector.tensor_copy(kT_sb[:, ot, tc4 * 512:(tc4 + 1) * 512], ps[:])

        _vdone = set()

        # V in token-major layout: lhsT = xT tile (stationary), rhs = w
        def vproj(tt):
            if tt in _vdone:
                return
            _vdone.add(tt)
            ps = ps_op.tile([128, 512], FP32, tag="op", name="ps")
            for k in range(KT):
                src = (
                    xtv0[:, k, tt * 128:(tt + 1) * 128]
                    if tt < 4
                    else xtvr[:, k, tt // 4 - 1, (tt % 4) * 128:(tt % 4 + 1) * 128]
                )
                nc.tensor.matmul(
                    ps[:],
                    src,
                    wv_sb[:, k, :],
                    start=(k == 0),
                    stop=(k == KT - 1),
                )
            nc.vector.tensor_copy(vp_sb[:, tt, :, 32:96], ps[:])

        # ---- attention ------------------------------------------------------
        fill_q = []

        def pop_fill(n, reserve=0):
            while n > 0 and len(fill_q) > reserve:
                fill_q.pop(0)()
                n -= 1

        def offof(qc, j):
            r = j - 4 * qc if causal else -1
            return 128 * r if r >= 0 else 0

        def scores(qc, hp, j):
            off = offof(qc, j)
            q0 = qc * 512
            pss = ps_s.tile([128, 2, 512], FP32, tag="ps_s", name="pss")
            for h2 in range(2):
                nc.tensor.matmul(
                    pss[:, h2, off:512],
                    kT_sb[h2 * 64:(h2 + 1) * 64, hp, j * 128:(j + 1) * 128],
                    qT_sb[h2 * 64:(h2 + 1) * 64, hp, q0 + off:q0 + 512],
                    start=True,
                    stop=True,
                )
            et = et_pool.tile([128, 2, 512], DTBF, tag="et", name="et")
            nc.scalar.activation(et[:, :, off:], pss[:, :, off:], EXP, scale=0.125)
            if off or (causal and j == 4 * qc):
                # zero where k (partition) > q (free col), both heads
                nc.vector.tensor_mul(
                    et[:, :, off:off + 128],
                    et[:, :, off:off + 128],
                    tri_sb[:],
                )
            return et

        def attn_finish(qc, hp, pso):
            # softmax denominators sit in pso[0:32]; reciprocal on DVE, then
            # normalize the two 32-row V halves straight into the o-proj
            # operand layout
            rc = rc_pool.tile([128, 2, 512], FP32, tag="rc", name="rc")
            for h2 in range(2):
                nc.vector.reciprocal_approx_fast(rc[0:32, h2, :], pso[0:32, h2, :])
                for q in range(2):
                    nc.vector.tensor_mul(
                        a_sbs[hp][
                            h2 * 64 + q * 32:h2 * 64 + (q + 1) * 32,
                            qc * 512:(qc + 1) * 512,
                        ],
                        pso[32 + q * 32:32 + (q + 1) * 32, h2, :],
                        rc[0:32, h2, :],
                    )

        def attn_av(qc, hp, j, jmax, pso, et):
            off = offof(qc, j)
            for h2 in range(2):
                # rows 0:32 accumulate the softmax denominator (ones block),
                # rows 32:96 attn@V.  Causally-trimmed widths on interleaved
                # chains; per-element has_written semantics make this safe but
                # the sim's zero-region tracker can't express it.
                nc.tensor.matmul(
                    pso[0:96, h2, off:512],
                    vp_sb[:, j, 2 * hp + h2, :],
                    et[:, h2, off:],
                    start=(j == 0),
                    stop=(j == jmax),
                    skip_group_check=True,
                )

        def attn(qc, hp, start_fills=2, reserve=0):
            jmax = 4 * qc + 3 if causal else TT - 1
            pso = ps_acc.tile([128, 2, 512], FP32, tag="acc", name="pso", bufs=1)
            kproj_chain(hp, 0)
            qproj_chain(hp, qc)
            et_next = scores(qc, hp, 0)
            pop_fill(start_fills, 0)
            for j in range(jmax + 1):
                et = et_next
                if j < jmax:
                    nj = j + 1
                    if nj % 4 == 0:
                        kproj_chain(hp, nj // 4)
                    et_next = scores(qc, hp, nj)
                # make sure the V tiles this and the next AV need are ahead
                # of the AV in the PE queue (prefetch ~2 ahead)
                vproj(j)
                if j + 2 <= jmax:
                    vproj(j + 2)
                attn_av(qc, hp, j, jmax, pso, et)
                if j % 2 == 1:
                    pop_fill(1, reserve)
            attn_finish(qc, hp, pso)

        def oproj_od(qc, od, ring_=None, ps_ap=None):
            if ps_ap is None:
                ps = ps_op.tile([128, 512], FP32, tag="op", name="ps")
                ps_ap = ps[:]
            for hp in range(NHP):
                nc.tensor.matmul(
                    ps_ap,
                    wo_sb[:, hp, od * 128:(od + 1) * 128],
                    a_sbs[hp][:, qc * 512:(qc + 1) * 512],
                    start=(hp == 0),
                    stop=(hp == NHP - 1),
                )
            osb = out_pool.tile([128, 512], DTBF, tag="osb", name="osb")
            nc.vector.tensor_scalar_add(osb[:], ps_ap, bo_sb[:, od:od + 1])
            (ring_ or nc.sync).dma_start(
                out_pt[od * 128:(od + 1) * 128, qc * 512:(qc + 1) * 512], osb[:]
            )

        if causal:
            # ---- warmup: units (0,0) and (0,1); (0,0)'s scores batched so
            # the exp stream starts as soon as the first k tier lands ------
            kproj_chain(0, 0)
            pso0 = ps_acc.tile([128, 2, 512], FP32, tag="acc", name="pso", bufs=1)
            ets0 = [scores(0, 0, j) for j in range(4)]
            kproj_chain(1, 0)
            kproj_chain(2, 0)
            kproj_chain(3, 0)
            for tt in range(4):
                vproj(tt)
            for j in range(4):
                attn_av(0, 0, j, 3, pso0, ets0[j])
            attn_finish(0, 0, pso0)
            attn(0, 1, start_fills=0)
            # kproj tier 1 must fully run here: the tier-2/3 k DMAs reuse the
            # two xtk slots and WAR-wait on the tier-0/1 reads
            kproj_chain(0, 1)
            kproj_chain(1, 1)
            kproj_chain(2, 1)
            kproj_chain(3, 1)

            # ---- middle: exp-heavy units while fill inventory (remaining
            # projections, then o-proj as each qc completes) keeps PE busy --
            fill_q.extend([
                lambda: vproj(4), lambda: vproj(5),
                lambda: vproj(6), lambda: vproj(7),
                lambda: kproj_chain(0, 2), lambda: kproj_chain(1, 2),
                lambda: vproj(8), lambda: vproj(9),
                lambda: vproj(10), lambda: vproj(11),
                lambda: kproj_chain(2, 2), lambda: kproj_chain(3, 2),
                lambda: kproj_chain(0, 3), lambda: kproj_chain(1, 3),
                lambda: vproj(12), lambda: vproj(13),
                lambda: vproj(14), lambda: vproj(15),
                lambda: kproj_chain(2, 3), lambda: kproj_chain(3, 3),
                lambda: qproj_chain(2, 3), lambda: qproj_chain(3, 3),
                lambda: qproj_chain(2, 2), lambda: qproj_chain(3, 2),
                lambda: qproj_chain(2, 1), lambda: qproj_chain(3, 1),
                lambda: qproj_chain(2, 0), lambda: qproj_chain(3, 0),
            ])
            order = [
                (1, 0), (2, 0), (3, 0), (2, 1), (3, 1), (1, 1),
                (3, 2), (2, 2), (3, 3), (1, 2), (2, 3), (0, 2),
                (1, 3), (0, 3),
            ]
            done_by_qc = {0: 2, 1: 0, 2: 0, 3: 0}
            for qc, hp in order:
                attn(qc, hp, start_fills=2, reserve=0)
                done_by_qc[qc] += 1
                if done_by_qc[qc] == 4 and qc != 0:
                    fill_q.extend(
                        lambda od=od, q=qc: oproj_od(q, od) for od in range(8)
                    )
            while fill_q:
                fill_q.pop(0)()
            qc_fin = 0
        else:
            # tier-major kproj: the 2-slot xtk rotation requires all four ot
            # chains of tier t to complete before tier t+2's DMA can land
            for tc4 in range(2):
                for ot in range(4):
                    kproj_chain(ot, tc4)
            for tc4 in range(4):
                qproj_chain(2, tc4)
            for tc4 in range(4):
                qproj_chain(3, tc4)
            for tt in range(TT):
                vproj(tt)
            for tc4 in range(2, 4):
                for ot in range(4):
                    kproj_chain(ot, tc4)
            for hp in range(NHP):
                attn(0, hp, start_fills=0, reserve=0)
            for qc in range(1, QC):
                fill_q.extend(
                    (lambda od=od: oproj_od(qc - 1, od)) for od in range(8)
                )
                sf = 3 if qc >= 2 else 2
                for hp in range(NHP):
                    attn(qc, hp, start_fills=sf, reserve=sf * (NHP - 1 - hp))
            while fill_q:
                fill_q.pop(0)()
            qc_fin = QC - 1

        # ---- final chunk's o-proj across 8 independent accumulators.  The
        # hp=0..2 partial accumulations only read already-normalized head
        # pairs, so they are emitted first and keep the PE busy while the
        # last unit's reciprocal+normalize drain; the hp=3 closers follow.
        fin = [ps_s.tile([128, 2, 512], FP32, tag="ps_s", name=f"fin{i}") for i in range(2)]
        fin_op = [ps_op.tile([128, 512], FP32, tag="op", name=f"fino{i}") for i in range(2)]
        fin_acc = ps_acc.tile([128, 2, 512], FP32, tag="acc", name="fin_acc", bufs=1)
        chains = [
            fin[0][:, 0, :], fin[0][:, 1, :], fin[1][:, 0, :], fin[1][:, 1, :],
            fin_op[0][:], fin_op[1][:], fin_acc[:, 0, :], fin_acc[:, 1, :],
        ]
        for od in range(8):
            for hp in range(NHP - 1):
                nc.tensor.matmul(
                    chains[od],
                    wo_sb[:, hp, od * 128:(od + 1) * 128],
                    a_sbs[hp][:, qc_fin * 512:(qc_fin + 1) * 512],
                    start=(hp == 0),
                    stop=False,
                )
        FIN_RINGS = [nc.sync, nc.gpsimd, nc.scalar, nc.sync]
        for od in range(8):
            nc.tensor.matmul(
                chains[od],
                wo_sb[:, NHP - 1, od * 128:(od + 1) * 128],
                a_sbs[NHP - 1][:, qc_fin * 512:(qc_fin + 1) * 512],
                start=False,
                stop=True,
            )
            osb = out_pool.tile([128, 512], DTBF, tag="osb", name="osb")
            # split the final bias-adds across DVE and the (now idle)
            # ScalarE so the tail evacuation doesn't serialize on one engine
            if od % 2 == 0:
                nc.vector.tensor_scalar_add(osb[:], chains[od], bo_sb[:, od:od + 1])
            else:
                nc.scalar.activation(
                    osb[:], chains[od],
                    mybir.ActivationFunctionType.Identity,
                    bias=bo_sb[:, od:od + 1],
                )
            FIN_RINGS[od % 4].dma_start(
                out_pt[od * 128:(od + 1) * 128, qc_fin * 512:(qc_fin + 1) * 512],
                osb[:],
            )


_CACHE = {}


def _get_compiled(causal: bool):
    key = bool(causal)
    if key not in _CACHE:
        nc = bacc.Bacc("TRN2", target_bir_lowering=False, debug=False, num_devices=NCORES)
        _emit(nc, causal=key)
        nc.compile()
        _CACHE[key] = nc
    return _CACHE[key]


def make_in_maps(query, key, value, w_q, b_q, w_k, b_k, w_v, b_v, w_o, b_o):
    """Build the per-core input maps (host-side sharding + layout prep)."""
    in_maps = []
    # b_v folds into the output bias: softmax rows sum to 1, so
    # attn(V + b_v) = attn(V) + b_v, and (A + b_v) @ w_o.T = A @ w_o.T + w_o @ b_v.
    # b_k drops entirely: scores shift constant along k cancels in softmax.
    bo_eff = (b_o + w_o.astype(np.float64) @ b_v.astype(np.float64)).astype(np.float32)
    for c in range(NCORES):
        b, hg = divmod(c, 2)
        sl = slice(hg * DL, (hg + 1) * DL)
        bo_core = bo_eff if hg == 0 else np.zeros_like(bo_eff)
        in_maps.append(
            {
                "xq_t": np.ascontiguousarray(query[b].T).astype(BF16),
                "xk_t": np.ascontiguousarray(key[b].T).astype(BF16),
                "xv_t": np.ascontiguousarray(value[b].T).astype(BF16),
                "wq_p": np.ascontiguousarray(
                    w_q[sl, :].T.reshape(KT, 128, DL).transpose(1, 0, 2)).astype(BF16),
                "wk_p": np.ascontiguousarray(
                    w_k[sl, :].T.reshape(KT, 128, DL).transpose(1, 0, 2)).astype(BF16),
                "wv_p": np.ascontiguousarray(
                    w_v[sl, :].T.reshape(KT, 128, DL).transpose(1, 0, 2)).astype(BF16),
                "wo_p": np.ascontiguousarray(
                    w_o[:, sl].T.reshape(NHP, 128, D).transpose(1, 0, 2)).astype(BF16),
                "bq_t": np.ascontiguousarray(b_q[sl].reshape(4, 128).T).astype(np.float32),
                "bo_t": np.ascontiguousarray(bo_core.reshape(8, 128).T).astype(np.float32),
            }
        )
    return in_maps


def _mask_is_causal(mask):
    m = np.asarray(mask).reshape(S, S)
    return bool(np.array_equal(m, np.triu(np.ones((S, S), bool), k=1)))


def _mask_is_empty(mask):
    return not np.asarray(mask).any()


def kernel(query, key, value, mask, w_q, b_q, w_k, b_k, w_v, b_v, w_o, b_o, **_unused):
    query = np.asarray(query, np.float32)
    key = np.asarray(key, np.float32)
    value = np.asarray(value, np.float32)
    if _mask_is_causal(mask):
        causal = True
    elif _mask_is_empty(mask):
        causal = False
    else:
        raise NotImplementedError("only causal or empty masks are supported")

    nc = _get_compiled(causal)
    in_maps = make_in_maps(
        query, key, value,
        np.asarray(w_q, np.float32), np.asarray(b_q, np.float32),
        np.asarray(w_k, np.float32), np.asarray(b_k, np.float32),
        np.asarray(w_v, np.float32), np.asarray(b_v, np.float32),
        np.asarray(w_o, np.float32), np.asarray(b_o, np.float32),
    )
    res = bass_utils.run_bass_kernel_spmd(nc, in_maps, core_ids=list(range(NCORES)))
    out = np.empty((B, S, D), np.float32)
    for b in range(B):
        acc = (
            res.results[2 * b]["out_pt"].astype(np.float32)
            + res.results[2 * b + 1]["out_pt"].astype(np.float32)
        )
        out[b] = acc.T
    return out
